# revision 58
# baseline (speedup 1.0000x reference)
"""Multi-head attention Trainium2 Bass kernel (v4.4). 197.3us
(v2 baseline: 208.9us; all numbers at full clock, min-MM 215ns).

v4.x over v3:
  - PSUM remap: scores ring = 2 slots (banks 0-3), O-proj owns banks
    4/5, PV owns 6/7.  O-proj never collides with PV, so the old
    hold/drain/dribble machinery shrank to a 1-unit pair-handoff hold
    + a 4-mt dribble; chunk-boundary PE bubbles fell ~3.3->1us/chunk
    (PE 94.5% busy in attention).  The cost: scores(u+1) waits
    exp(u) directly (2-slot ring) - with 8 banks you get scores
    lookahead OR O-proj isolation, not both.
  - Projection interleave: only K, the chunk-0 Q chains, and 4 V
    tiles run before attention; V tts 4-15 and Q chains c1-3 dribble
    into chunk-0's units as PE filler, accumulating in the then-idle
    O-proj banks.  First scores at 32us instead of 56us; the
    attention-start and chunk-0/1 boundary gaps vanished.
  - PE prewarm (20 junk matmuls) + exp rebalance (2 units/chunk with
    both slots exact on ScalarE, at in_chunk {4,10} - load-bearing
    placement) + final-chunk O-proj over all 4 bank pairs with
    phase-0 pair-0 matmuls and junk bridges riding the closing norm
    latency (burst measured warm at 216ns/MM) + 2-way split of only
    the last out-DMA.
Measured dead ends (reverted): fp8/DoubleRow anywhere (e4m3 noise
blows the 2e-2 gate: PV 1.9e-2, proj 3.0e-2, O-proj 2.8e-2 emulated);
2048-wide merged exp ops (couple the two PSUM slots -> ring stalls,
+28us); per-unit engine alternation for exp (serializes slot frees,
+22us); GpSimd Newton chain (1.15us/op serial chain parks an O-proj
matmul at the PE queue head ~2.3us/chunk); splitting DMAs across
queues at the tail (each dma_start is a ~600ns serial Sync-queue
DIRECT2D); splitting the first x-tile DMAs (descriptor inflation
starves the K-projection); scalar_tensor_tensor Newton fusion (STT
is 1x-mode-only: 691ns vs the 293/327ns 2x ts/tt it replaces);
double-S exp units in the drain region (+48us incl thermal).

Bench note: back-to-back runs heat the chip into P0 (PE 2.4->2.0
GHz, warm N=512 matmul 215->258ns, ~+17% exec).  Compare only runs
with matching min-matmul durations.

Remaining structural gap (~190us floor): the chunk-boundary bubbles
(~3.3us/chunk) come from 8-bank PSUM scarcity - scores need 6, PV 2,
and O-proj reuses the PV pair, serializing norm -> O-proj -> PV
behind per-bank evicts.  A fix needs 10 banks or 512-query chunks.

Problem: B=2, T=2048, D=1024, H=16 heads, dk=64 (fp32).
  out = softmax((x@Wq.T+bq)(x@Wk.T+bk).T / 8) (x@Wv.T+bv) @ Wo.T + bo

Sharding (8 cores): data-parallel over B (2) x tensor-parallel over 4
head-groups of 4 heads.  Core (b, g) computes, for batch b and heads
[4g, 4g+4): Q/K/V projections (column-sliced Wq/Wk/Wv), attention, and
the row-sliced Wo projection, producing a partial (2048, 1024) fp16
output.  Host sums the partials per batch in fp32 and adds the bias
terms.

Bias algebra (removes all device-side bias work except bq):
  - bk shifts every score of a query by a constant -> softmax-invariant
    -> dropped entirely.
  - bv: softmax rows sum to 1, so the bv contribution to the output is
    the constant row bv @ Wo.T -> folded into bo on the host.
  - bq: added on the Q-projection eviction via a per-partition
    tensor_scalar add (Q.T layout has features on partitions).

Per-core device schedule (everything fp16 operands, fp32 PSUM):
  - One persistent PSUM tensor sf [128, 4096] (all 8 banks) managed
    manually with subtile dependency tracking - no pool barriers, so
    the scheduler freely overlaps phases.
  - Projections (k-outer, 8 full-bank chains): K.T -> V -> Q.T, each
    chain accumulates 8 k-tiles; DMAs are issued in consumption order
    so the PE starts as soon as wk0+xt0 land.
  - V stored as V_aug [128, 16*384]: per key-tile, per head-pair block
    [V_even|ones64|V_odd] so the PV matmul also produces the softmax
    denominator (replicated across 64 partitions) for free.
  - Attention per (chunk c of 512 queries, head-pair):  scores.T tiles
    [128 keys, 512q] per head, both heads of the pair packed into one
    1024-wide PSUM slot (row-group-concurrent matmuls, contraction 64).
    3 slots (banks 0-5) rotate; ScalarE exp's TWO slots per ACTIVATE
    (2048 wide, via a 3D AP, negative-stride for the wrap pattern) to
    amortize the ~313-cycle ACT overhead.  PV accumulates in banks 6-7.
  - Normalization: denominators evicted to fp16 SBUF; 1/d via int16
    magic-subtract seed + one fp16 Newton step (beats the DVE's 8
    cycle/element iterative reciprocal ~3x); O * (1/d) in fp16.
  - Output projection accumulates head-pairs in banks 6/7 (after PV is
    evicted), evicts fp16, DMAs fp16 partials out (halves DMA bytes).
"""

import numpy as np

D = 1024          # d_model
T = 2048          # sequence length
G = 256           # features per head-group (4 heads * 64)
DK = 64
NKT = D // 128    # 8 contraction tiles for projections
NTT = T // 128    # 16 key tiles
NCH = T // 512    # 4 query chunks of 512
VROW = 2 * 192    # V_aug row per key tile: 2 blocks of [V_e|ones64|V_o]
MAGIC = 0x7798    # fp16 reciprocal seed: bitcast(MAGIC - bits16(d))
# fp16 Schraudolph exp for the DVE half: bitcast16(rint(s*EXP_A + EXP_B))
# ~= exp(s/8), max rel err ~3% pointwise, ~6.5e-3 end-to-end (softmax
# weights are consistent: the denominator sums the same approximated p).
EXP_A = 0.125 * 1.4426950408889634 * 1024.0
EXP_B = 15360.0 - 44.5

_CACHE = {}


def _split_multi_waits(nc):
    """walrus's TRN2 codegen rejects >1 sync-wait on datapath instruction
    structs.  Hoist every wait of a multi-wait datapath instruction onto
    single-wait NoOps just before it on the same engine queue."""
    import concourse.mybir as mybir

    keep = ("InstEventSemaphore", "InstUnconditionalBranch",
            "InstCall", "InstBranchHint", "InstHalt", "InstNoOp",
            "InstAllEngineBarrier", "InstCompareAndBranch")
    nid = [0]
    for f in nc.m.functions:
        for bb in f.blocks:
            new = []
            for ins in bb.instructions:
                si = ins.sync_info
                waits = list(si.on_wait) if si and si.on_wait else []
                if len(waits) >= 2 and type(ins).__name__ not in keep:
                    for w in waits:
                        nid[0] += 1
                        nop = mybir.InstNoOp(name=f"{ins.name}-wsplit{nid[0]}",
                                             ins=[], outs=[])
                        nop.engine = ins.engine
                        nop.sync_info = mybir.SyncInfo(on_wait=[w], on_update=[])
                        new.append(nop)
                    ins.sync_info = mybir.SyncInfo(
                        on_wait=[], on_update=list(si.on_update or []))
                new.append(ins)
            bb.instructions = new
    return nc


def _build(split_waits=True):
    import concourse.bass as bass
    import concourse.mybir as mybir
    import concourse.tile as tile
    import bass_rust

    f32 = mybir.dt.float32
    f16 = mybir.dt.float16
    i16 = mybir.dt.int16
    ALU = mybir.AluOpType
    EXP = mybir.ActivationFunctionType.Exp
    CPY = mybir.ActivationFunctionType.Copy
    nc = bass.Bass()

    xT = nc.dram_tensor("xT", [D, T], f16, kind="ExternalInput")
    wqT = nc.dram_tensor("wqT", [D, G], f16, kind="ExternalInput")
    wkT = nc.dram_tensor("wkT", [D, G], f16, kind="ExternalInput")
    wvT = nc.dram_tensor("wvT", [D, G], f16, kind="ExternalInput")
    woT = nc.dram_tensor("woT", [G, D], f16, kind="ExternalInput")
    bqc = nc.dram_tensor("bqc", [128, 2], f32, kind="ExternalInput")
    out = nc.dram_tensor("out", [T, D], f16, kind="ExternalOutput")

    with tile.TileContext(nc) as tc:
        with tc.tile_pool(name="sb", bufs=1) as sb, \
             tc.tile_pool(name="dyn", bufs=2) as dyn, \
             tc.tile_pool(name="ps", bufs=1, space="PSUM") as ps:

            # ---- the one PSUM tensor: 8 banks, manual ranges ----
            sf = ps.tile([128, 4096], f32, tag="sf", name="sf")

            # ---- PE prewarm: junk matmuls during the DMA wait keep the
            # HAM activity window busy so the real projections start at
            # 2.4 GHz instead of paying ~3.4us of 1.2 GHz cold ramp.
            # They write the pv area (banks 6/7), which the first real PV
            # overwrites with start=True.
            junk = sb.tile([128, 256], f16, tag="junk", name="junk")
            nc.gpsimd.memset(junk, 0.5)
            for _ in range(16):
                nc.tensor.matmul(out=sf[:, 3072:3328], lhsT=junk[:, 0:128],
                                 rhs=junk, start=True, stop=True)

            # ---- DMAs in consumption order ----
            wk_sb, xt = [], []
            for k in range(NKT):
                t = sb.tile([128, G], f16, tag=f"wk{k}", name=f"wk{k}")
                nc.sync.dma_start(out=t, in_=wkT[k * 128:(k + 1) * 128, :])
                wk_sb.append(t)
                t = sb.tile([128, T], f16, tag=f"xt{k}", name=f"xt{k}")
                nc.sync.dma_start(out=t, in_=xT[k * 128:(k + 1) * 128, :])
                xt.append(t)
            bq_sb = sb.tile([128, 2], f32, tag="bq", name="bq_sb")
            nc.sync.dma_start(out=bq_sb, in_=bqc[:, :])
            # warm the ScalarE exp table-set (~2.7us) during the DMA wait
            scr = sb.tile([128, 2], f16, tag="scr", name="scr")
            nc.scalar.activation(out=scr, in_=bq_sb, func=EXP, scale=0.0)
            # wq before wv: the chunk-0 Q chains run right after K so the
            # attention (and its exp streams) can start ~14us earlier
            wv_sb, wq_sb = [], []
            for nm, dram, lst in (("wq", wqT, wq_sb), ("wv", wvT, wv_sb)):
                for k in range(NKT):
                    t = sb.tile([128, G], f16, tag=f"{nm}{k}", name=f"{nm}{k}")
                    nc.sync.dma_start(out=t, in_=dram[k * 128:(k + 1) * 128, :])
                    lst.append(t)
            wo_sb = []
            for p2 in range(2):
                t = sb.tile([128, D], f16, tag=f"wo{p2}", name=f"wo{p2}")
                nc.sync.dma_start(out=t, in_=woT[p2 * 128:(p2 + 1) * 128, :])
                wo_sb.append(t)

            # ---- persistent SBUF ----
            qt = [sb.tile([128, T], f16, tag=f"qt{p}", name=f"qt{p}")
                  for p in range(2)]
            kt = [sb.tile([128, T], f16, tag=f"kt{p}", name=f"kt{p}")
                  for p in range(2)]
            va = sb.tile([128, NTT * VROW], f16, tag="va", name="va")
            va6 = va.rearrange("p (t b x) -> p t b x", t=NTT, b=6)
            nc.vector.memset(va6[:, :, 1::3, :], 1.0)   # ones64 columns

            # PSUM map: scores ring = 2 slots (banks 0-3), O-proj has its
            # OWN bank pair (4/5) so it never collides with PV (6/7).
            # The 3-slot ring gave ~1.5 units of lookahead but forced
            # O-proj to time-multiplex the PV banks, serializing
            # norm -> O-proj -> PV behind per-bank evicts at every chunk
            # boundary (~3.3us/chunk of PE bubbles).
            sf3 = sf[:, 0:3072].rearrange("p (s x) -> p s x", s=3)
            OPB = 2048          # O-proj bank pair base column
            pv_e = sf[:, 3072:3584]
            pv_o = sf[:, 3584:4096]

            def chain(i):       # 8 full-bank projection chains
                return sf[:, i * 512:(i + 1) * 512]

            # ---- K.T projection: chains (p2, c), k-outer ----
            for k in range(NKT):
                for i in range(8):
                    p2, c = divmod(i, 4)
                    nc.tensor.matmul(
                        out=chain(i),
                        lhsT=wk_sb[k][:, p2 * 128:(p2 + 1) * 128],
                        rhs=xt[k][:, c * 512:(c + 1) * 512],
                        start=(k == 0), stop=(k == NKT - 1))
            for i in range(8):
                p2, c = divmod(i, 4)
                nc.vector.tensor_copy(
                    out=kt[p2][:, c * 512:(c + 1) * 512], in_=chain(i))

            # ---- V and Q projections: only chunk-0's two Q chains and
            # the first 4 V tiles run before attention; the rest dribble
            # into chunk-0's attention units as PE filler, accumulating
            # in the O-proj banks (4/5), which chunk-0 never touches
            # (chunk-1's O-proj dribble starts at unit 17).  This starts
            # the exp streams ~14us earlier and converts chunk-0's
            # 2-slot-ring stalls into projection work.
            va5 = va.rearrange("p (t pr b x) -> p t pr b x", t=NTT, pr=2, b=3)

            def emit_qchain(c, p2, base):
                for k in range(NKT):
                    nc.tensor.matmul(
                        out=sf[:, base:base + 512],
                        lhsT=wq_sb[k][:, p2 * 128:(p2 + 1) * 128],
                        rhs=xt[k][:, c * 512:(c + 1) * 512],
                        start=(k == 0), stop=(k == NKT - 1))
                nc.vector.tensor_scalar(
                    out=qt[p2][:, c * 512:(c + 1) * 512],
                    in0=sf[:, base:base + 512],
                    scalar1=bq_sb[:, p2:p2 + 1], scalar2=None, op0=ALU.add)

            def emit_vtt(tt, base):
                for k in range(NKT):
                    nc.tensor.matmul(
                        out=sf[:, base:base + G],
                        lhsT=xt[k][:, tt * 128:(tt + 1) * 128],
                        rhs=wv_sb[k][:, :],
                        start=(k == 0), stop=(k == NKT - 1))
                nc.vector.tensor_copy(
                    out=va5[:, tt, :, 0::2, :],
                    in_=sf[:, base:base + 256].rearrange(
                        "p (pr h x) -> p pr h x", pr=2, h=2))

            emit_qchain(0, 0, 0)
            emit_qchain(0, 1, 512)
            for tt in range(4):
                emit_vtt(tt, OPB + (tt % 4) * 256)

            def emit_filler(u):
                # V tts 4-15 at units 0-5 (2-unit lead over their PV
                # consumers), Q chains c1-3 at units 6-11 (5+ units
                # before chunk 1 needs qt)
                if u < 6:
                    for tt in (4 + 2 * u, 5 + 2 * u):
                        emit_vtt(tt, OPB + (tt % 4) * 256)
                elif u < 12:
                    i = u - 6
                    emit_qchain(1 + i // 2, i % 2, OPB + (i % 2) * 512)

            # ---- attention + output projection ----
            # Flat software pipeline over 64 "units" (one unit = 2 key
            # tiles of one (chunk, head-pair)).  Per-engine queues are
            # strict FIFO, so emission order IS the schedule skeleton:
            # scores(u+1) must be emitted BEFORE pv(u) (which waits on
            # exp(u)) or the PE queue head blocks and the whole loop
            # serializes; O-proj is emitted 2 units after its chunk ends
            # so the DVE normalization latency is hidden.
            units = [(c, pair, g) for c in range(NCH) for pair in range(2)
                     for g in range(8)]
            onorm = {}          # (c, pair) -> normalized O tile

            # Dynamic scores-slot schedule: slots 0/1 live in banks 0-3;
            # banks 4/5 (the O-proj pair) double as a THIRD slot during
            # the units whose write->exp-read lifetime avoids the O-proj
            # dribble windows (in_chunk ~1-3.5 and ~8-10.5, plus chunk
            # 0's projection fillers through unit ~12).  Greedy LRU picks
            # the two stalest allowed slots per unit, deepening the ring
            # from distance-1 wherever slot 2 is usable.
            CB = {0: 0, 1: 1024, 2: OPB}
            SLOTS = []
            _last = {0: -3.0, 1: -2.0, 2: -1.0}
            for _u in range(len(units)):
                _c, _ic = _u // 16, _u % 16
                if _c == 0:
                    _ok2 = _ic in (13, 14)
                else:
                    _ok2 = _ic in (5, 6, 7, 12, 13, 14)
                _allowed = (0, 1, 2) if _ok2 else (0, 1)
                _pick = sorted(_allowed, key=lambda s: _last[s])[:2]
                SLOTS.append((_pick[0], _pick[1]))
                _last[_pick[0]] = _u
                _last[_pick[1]] = _u + 0.5

            def emit_scores(u):
                c, pair, g = units[u]
                cs = slice(c * 512, (c + 1) * 512)
                sA, sB = SLOTS[u]
                for tk, s in ((2 * g, sA), (2 * g + 1, sB)):
                    for h in range(2):   # packed row-group pair
                        nc.tensor.matmul(
                            out=sf[:, s * 1024 + h * 512:
                                   s * 1024 + (h + 1) * 512],
                            lhsT=kt[pair][h * 64:(h + 1) * 64,
                                          tk * 128:(tk + 1) * 128],
                            rhs=qt[pair][h * 64:(h + 1) * 64, cs],
                            start=True, stop=True)

            def emit_exp(u):
                # tkA: exact exp on ScalarE; tkB: Schraudolph fast-exp on
                # the DVE (one fused mul-add into int16, bitcast to fp16).
                # The two slots live in different PSUM banks, so the two
                # engines stream concurrently - the slot-pair latency is
                # max(S, V), which is what keeps the 3-slot ring moving.
                c, pair, g = units[u]
                sA, sB = SLOTS[u]
                pa = dyn.tile([128, 1024], f16, tag="pa", bufs=12,
                              name=f"pa_{c}_{pair}_{g}")
                nc.scalar.activation(out=pa, in_=sf3[:, sA, :], func=EXP,
                                     scale=0.125)
                if u % 16 in (4, 10):
                    # rebalance: the DVE also carries the Newton chains
                    # and half the evicts, so 2 units per chunk put BOTH
                    # halves on ScalarE (exact exp - also trims the
                    # Schraudolph error a little).  The {4,10} placement
                    # is load-bearing: moving these to the drain region
                    # {8,12} measured +48us - the held-PV bursts at
                    # in_chunk 7-9 are timed against single-exp pa
                    # completion there.
                    pb2 = dyn.tile([128, 1024], f16, tag="pbS", bufs=3,
                                   name=f"pbS_{c}_{pair}_{g}")
                    nc.scalar.activation(out=pb2, in_=sf3[:, sB, :],
                                         func=EXP, scale=0.125)
                    return pa, pb2
                pb = dyn.tile([128, 1024], i16, tag="pb", bufs=12,
                              name=f"pb_{c}_{pair}_{g}")
                nc.vector.tensor_scalar(
                    out=pb, in0=sf3[:, sB, :], scalar1=EXP_A, scalar2=EXP_B,
                    op0=ALU.mult, op1=ALU.add)
                return pa, pb.bitcast(f16)

            def emit_pv(u, pab):
                c, pair, g = units[u]
                for j in range(2):
                    tk = 2 * g + j
                    off = tk * VROW + pair * 192
                    nc.tensor.matmul(
                        out=pv_e, lhsT=va[:, off:off + 128],
                        rhs=pab[j][:, 0:512],
                        start=(tk == 0), stop=(tk == NTT - 1))
                    nc.tensor.matmul(
                        out=pv_o, lhsT=va[:, off + 64:off + 192],
                        rhs=pab[j][:, 512:1024],
                        start=(tk == 0), stop=(tk == NTT - 1))

            def emit_norm(c, pair):
                # pv_e = [O_e; d_e], pv_o = [d_o; O_o]; 1/d via int16
                # magic seed + one fp16 Newton step.  Base-aligned O
                # evicts on ScalarE, cross-base denominator evicts on the
                # DVE (both need the PSUM port).  The magic subtract is
                # SBUF-only, so it goes to the otherwise-idle GpSimd for
                # chunks 0-2; the last chunk keeps it on ScalarE so the
                # tail latency stays short.
                oo = dyn.tile([128, 512], f16, tag="oo", name=f"oo{c}{pair}")
                dd = dyn.tile([128, 512], f16, tag="dd", name=f"dd{c}{pair}")
                nc.scalar.activation(out=oo[0:64, :], in_=pv_e[0:64, :],
                                     func=CPY)
                nc.scalar.activation(out=oo[64:128, :], in_=pv_o[64:128, :],
                                     func=CPY)
                nc.vector.tensor_copy(out=dd[0:64, :], in_=pv_e[64:128, :])
                nc.vector.tensor_copy(out=dd[64:128, :], in_=pv_o[0:64, :])
                r0 = dyn.tile([128, 512], i16, tag="r0", name=f"r0{c}{pair}")
                nc.scalar.activation(out=r0, in_=dd.bitcast(i16),
                                     func=CPY, scale=-1.0,
                                     bias=float(MAGIC))
                norm_st[(c, pair)] = (oo, dd, r0)

            def emit_norm2(c, pair):
                # Newton step + final multiply on the DVE.  (Both the
                # gpsimd offload and a scalar_tensor_tensor fusion were
                # tried and lost: gpsimd's 1.15us/op serial chain and
                # STT's 1x-only mode (691ns vs 293/327 for ts/tt) are
                # slower than this 4-op 2x-mode chain.)
                oo, dd, r0 = norm_st.pop((c, pair))
                eng = nc.vector
                r = r0.bitcast(f16)
                tn = dyn.tile([128, 512], f16, tag="tn", name=f"tn{c}{pair}")
                eng.tensor_tensor(out=tn, in0=dd, in1=r, op=ALU.mult)
                un = dyn.tile([128, 512], f16, tag="un", name=f"un{c}{pair}")
                eng.tensor_scalar(
                    out=un, in0=tn, scalar1=-1.0, scalar2=2.0,
                    op0=ALU.mult, op1=ALU.add)
                r1 = dyn.tile([128, 512], f16, tag="r1", name=f"r1{c}{pair}")
                eng.tensor_tensor(out=r1, in0=r, in1=un, op=ALU.mult)
                on = dyn.tile([128, 512], f16, tag=f"on{pair}",
                              name=f"on{c}{pair}")
                eng.tensor_tensor(out=on, in0=oo, in1=r1, op=ALU.mult)
                onorm[(c, pair)] = on

            def emit_oproj(c, mts=range(4), final=False, phase=None):
                # Mid-kernel: banks 6/7 only (scores own 0-5); per-512
                # evicts alternate engines so the next 512-slot's matmuls
                # never wait a both-bank evict.  Final chunk: scores are
                # done, so each mt gets its own bank PAIR - all 16 matmuls
                # run back-to-back and the [128,1024] evicts + full-row
                # DMAs pipeline behind them.  phase=0 emits only the
                # pair-0 accumulation matmuls (they ride the pair-1 norm
                # latency); phase=1 closes with pair-1 + evict + DMA.
                for mt in mts:
                    base = mt * 1024 if final else OPB
                    for n2 in range(2):
                        ops = sf[:, base + n2 * 512:base + (n2 + 1) * 512]
                        prs = (0, 1) if phase is None else (phase,)
                        for pair in prs:
                            nc.tensor.matmul(
                                out=ops,
                                lhsT=onorm[(c, pair)][:, mt * 128:(mt + 1) * 128],
                                rhs=wo_sb[pair][:, n2 * 512:(n2 + 1) * 512],
                                start=(pair == 0), stop=(pair == 1))
                        if phase == 0:
                            continue
                        if not final:
                            osb = dyn.tile([128, 512], f16, tag="osb",
                                           bufs=4, name=f"osb_{c}_{mt}_{n2}")
                            if n2 == 0:
                                nc.scalar.activation(out=osb, in_=ops,
                                                     func=CPY)
                            else:
                                nc.vector.tensor_copy(out=osb, in_=ops)
                            nc.sync.dma_start(
                                out=out[c * 512 + mt * 128:
                                        c * 512 + (mt + 1) * 128,
                                        n2 * 512:(n2 + 1) * 512],
                                in_=osb)
                    if final and phase != 0:
                        osb = dyn.tile([128, 1024], f16, tag="osbf",
                                       bufs=4, name=f"osbf_{c}_{mt}")
                        if mt % 2 == 0:
                            nc.scalar.activation(
                                out=osb, in_=sf[:, base:base + 1024],
                                func=CPY)
                        else:
                            nc.vector.tensor_copy(
                                out=osb, in_=sf[:, base:base + 1024])
                        # one dma_start per mt (each is a ~600ns serial
                        # DIRECT2D on the Sync queue) - except the LAST
                        # tile, whose drain IS the kernel end: 2-way
                        # split halves its single-queue descriptor drain
                        if mt == 3:
                            nc.sync.dma_start(
                                out=out[c * 512 + mt * 128:
                                        c * 512 + mt * 128 + 64, :],
                                in_=osb[0:64, :])
                            nc.sync.dma_start(
                                out=out[c * 512 + mt * 128 + 64:
                                        c * 512 + (mt + 1) * 128, :],
                                in_=osb[64:128, :])
                        else:
                            nc.sync.dma_start(
                                out=out[c * 512 + mt * 128:
                                        c * 512 + (mt + 1) * 128, :],
                                in_=osb)

            # Emission = per-engine FIFO order.  Skews:
            #  - scores/exp of u+1 before pv(u), so the PE computes the
            #    next unit's scores while ScalarE exp's unit u.
            #  - With O-proj on its own bank pair, only the PAIR handoff
            #    needs a hold: each pair's first PV (start=True on banks
            #    6/7) is delayed one unit so the previous pair's oo/dd
            #    evicts clear in shadow.
            #  - O-proj(c-1) mts dribble into the two transition bubbles
            #    (chunk entry in_chunk 1-2, pair handoff 8-9) as PE
            #    filler.
            pas = {}
            pv_hold = []
            norm_st = {}
            norm_due = None
            norm2_due = None
            emit_scores(0)
            pas[0] = emit_exp(0)
            for u in range(len(units)):
                if u + 1 < len(units):
                    emit_scores(u + 1)
                    pas[u + 1] = emit_exp(u + 1)
                if norm2_due is not None:
                    # Newton chain one unit after the evicts: each engine's
                    # FIFO carries only a slice of the norm between exps
                    emit_norm2(*norm2_due)
                    norm2_due = None
                if norm_due is not None and norm_due[2] not in pas:
                    # skew the normalization late (and always after its
                    # pair's closing pv) so its ScalarE/DVE ops never block
                    # the exp stream at the queue head
                    emit_norm(norm_due[0], norm_due[1])
                    norm2_due = (norm_due[0], norm_due[1])
                    norm_due = None
                c, pair, g = units[u]
                in_chunk = u % 16
                if c > 0 and in_chunk in (1, 2, 8, 9):
                    emit_oproj(c - 1, mts=[{1: 0, 2: 1, 8: 2, 9: 3}[in_chunk]])
                if g == 0 and u >= 2:
                    pv_hold.append(u)
                else:
                    for uh in pv_hold:
                        emit_pv(uh, pas.pop(uh))
                    pv_hold = []
                    emit_pv(u, pas.pop(u))
                emit_filler(u)
                if g == 7:
                    norm_due = (c, pair, u)
            if norm2_due is not None:
                emit_norm2(*norm2_due)
            emit_norm(norm_due[0], norm_due[1])
            # keep the PE's HAM activity window busy through the ~2us
            # final-norm latency so the closing O-proj burst runs at
            # 2.4 GHz (it measured 427ns/MM = 1.2 GHz when the idle gap
            # crossed the re-throttle window)...
            for _ in range(4):
                nc.tensor.matmul(out=sf[:, 1024:1280], lhsT=junk[:, 0:128],
                                 rhs=junk, start=True, stop=True)
            # ...and fill the rest of that window with REAL work: the
            # pair-0 O-proj accumulation for mts 0-2 (banks 0-5; mt3 is
            # on the PV banks, blocked until pair-1's oo/dd)
            emit_oproj(NCH - 1, mts=range(3), final=True, phase=0)
            # bridge the remaining norm-wait idle so HAM stays at 8/8
            # (the closing burst measured 427ns/MM = 1.2 GHz without
            # this).  Target the PV banks: ordered AFTER the oo/dd
            # evicts (WAR) and BEFORE mt3's start=True overwrite - NOT
            # banks 0-5, which hold the phase-0 partial sums.
            for _ in range(10):
                nc.tensor.matmul(out=sf[:, 3072:3328], lhsT=junk[:, 0:128],
                                 rhs=junk, start=True, stop=True)
            emit_norm2(norm_due[0], norm_due[1])
            emit_oproj(NCH - 1, mts=range(3), final=True, phase=1)
            emit_oproj(NCH - 1, mts=[3], final=True)
    if split_waits:
        _split_multi_waits(nc)
    return nc


def _get_nc(split_waits=True):
    key = ("nc", split_waits)
    if key not in _CACHE:
        _CACHE[key] = _build(split_waits)
    return _CACHE[key]


def make_in_maps(x, Wq, bq, Wk, bk, Wv, bv, Wo):
    dt = np.float16
    in_maps = []
    for core in range(8):
        b, g = divmod(core, 4)
        gs = slice(g * G, (g + 1) * G)
        in_maps.append({
            "xT": np.ascontiguousarray(x[b].T).astype(dt),
            "wqT": np.ascontiguousarray(Wq[gs, :].T).astype(dt),
            "wkT": np.ascontiguousarray(Wk[gs, :].T).astype(dt),
            "wvT": np.ascontiguousarray(Wv[gs, :].T).astype(dt),
            "woT": np.ascontiguousarray(Wo[:, gs].T).astype(dt),
            "bqc": np.ascontiguousarray(bq[gs].reshape(2, 128).T).astype(np.float32),
        })
    return in_maps


def host_out_init(bo, bv, Wo):
    """bo + bv @ Wo.T (the bv contribution is exact: softmax rows sum to 1)."""
    return (bo.astype(np.float64)
            + bv.astype(np.float64) @ Wo.T.astype(np.float64)).astype(np.float32)


def kernel(x, Wq, bq, Wk, bk, Wv, bv, Wo, bo):
    from concourse.bass_utils import run_bass_kernel_spmd

    x = np.asarray(x, dtype=np.float32)
    Wq = np.asarray(Wq, dtype=np.float32)
    Wk = np.asarray(Wk, dtype=np.float32)
    Wv = np.asarray(Wv, dtype=np.float32)
    Wo = np.asarray(Wo, dtype=np.float32)
    bq = np.asarray(bq, dtype=np.float32)
    bv = np.asarray(bv, dtype=np.float32)
    bo = np.asarray(bo, dtype=np.float32)

    nc = _get_nc()
    in_maps = make_in_maps(x, Wq, bq, Wk, bk, Wv, bv, Wo)

    res = run_bass_kernel_spmd(nc, in_maps, core_ids=list(range(8)))
    outp = np.tile(host_out_init(bo, bv, Wo)[None, None, :], (2, T, 1))
    for core in range(8):
        outp[core // 4] += res.results[core]["out"].astype(np.float32)
    return outp



# revision 61
# speedup vs baseline: 1.0153x; 1.0153x over previous
"""Multi-head attention Trainium2 Bass kernel (v4.4). 197.3us
(v2 baseline: 208.9us; all numbers at full clock, min-MM 215ns).

v4.x over v3:
  - PSUM remap: scores ring = 2 slots (banks 0-3), O-proj owns banks
    4/5, PV owns 6/7.  O-proj never collides with PV, so the old
    hold/drain/dribble machinery shrank to a 1-unit pair-handoff hold
    + a 4-mt dribble; chunk-boundary PE bubbles fell ~3.3->1us/chunk
    (PE 94.5% busy in attention).  The cost: scores(u+1) waits
    exp(u) directly (2-slot ring) - with 8 banks you get scores
    lookahead OR O-proj isolation, not both.
  - Projection interleave: only K, the chunk-0 Q chains, and 4 V
    tiles run before attention; V tts 4-15 and Q chains c1-3 dribble
    into chunk-0's units as PE filler, accumulating in the then-idle
    O-proj banks.  First scores at 32us instead of 56us; the
    attention-start and chunk-0/1 boundary gaps vanished.
  - PE prewarm (20 junk matmuls) + exp rebalance (2 units/chunk with
    both slots exact on ScalarE, at in_chunk {4,10} - load-bearing
    placement) + final-chunk O-proj over all 4 bank pairs with
    phase-0 pair-0 matmuls and junk bridges riding the closing norm
    latency (burst measured warm at 216ns/MM) + 2-way split of only
    the last out-DMA.
Measured dead ends (reverted): fp8/DoubleRow anywhere (e4m3 noise
blows the 2e-2 gate: PV 1.9e-2, proj 3.0e-2, O-proj 2.8e-2 emulated);
2048-wide merged exp ops (couple the two PSUM slots -> ring stalls,
+28us); per-unit engine alternation for exp (serializes slot frees,
+22us); GpSimd Newton chain (1.15us/op serial chain parks an O-proj
matmul at the PE queue head ~2.3us/chunk); splitting DMAs across
queues at the tail (each dma_start is a ~600ns serial Sync-queue
DIRECT2D); splitting the first x-tile DMAs (descriptor inflation
starves the K-projection); scalar_tensor_tensor Newton fusion (STT
is 1x-mode-only: 691ns vs the 293/327ns 2x ts/tt it replaces);
double-S exp units in the drain region (+48us incl thermal).

Bench note: back-to-back runs heat the chip into P0 (PE 2.4->2.0
GHz, warm N=512 matmul 215->258ns, ~+17% exec).  Compare only runs
with matching min-matmul durations.

Remaining structural gap (~190us floor): the chunk-boundary bubbles
(~3.3us/chunk) come from 8-bank PSUM scarcity - scores need 6, PV 2,
and O-proj reuses the PV pair, serializing norm -> O-proj -> PV
behind per-bank evicts.  A fix needs 10 banks or 512-query chunks.

Problem: B=2, T=2048, D=1024, H=16 heads, dk=64 (fp32).
  out = softmax((x@Wq.T+bq)(x@Wk.T+bk).T / 8) (x@Wv.T+bv) @ Wo.T + bo

Sharding (8 cores): data-parallel over B (2) x tensor-parallel over 4
head-groups of 4 heads.  Core (b, g) computes, for batch b and heads
[4g, 4g+4): Q/K/V projections (column-sliced Wq/Wk/Wv), attention, and
the row-sliced Wo projection, producing a partial (2048, 1024) fp16
output.  Host sums the partials per batch in fp32 and adds the bias
terms.

Bias algebra (removes all device-side bias work except bq):
  - bk shifts every score of a query by a constant -> softmax-invariant
    -> dropped entirely.
  - bv: softmax rows sum to 1, so the bv contribution to the output is
    the constant row bv @ Wo.T -> folded into bo on the host.
  - bq: added on the Q-projection eviction via a per-partition
    tensor_scalar add (Q.T layout has features on partitions).

Per-core device schedule (everything fp16 operands, fp32 PSUM):
  - One persistent PSUM tensor sf [128, 4096] (all 8 banks) managed
    manually with subtile dependency tracking - no pool barriers, so
    the scheduler freely overlaps phases.
  - Projections (k-outer, 8 full-bank chains): K.T -> V -> Q.T, each
    chain accumulates 8 k-tiles; DMAs are issued in consumption order
    so the PE starts as soon as wk0+xt0 land.
  - V stored as V_aug [128, 16*384]: per key-tile, per head-pair block
    [V_even|ones64|V_odd] so the PV matmul also produces the softmax
    denominator (replicated across 64 partitions) for free.
  - Attention per (chunk c of 512 queries, head-pair):  scores.T tiles
    [128 keys, 512q] per head, both heads of the pair packed into one
    1024-wide PSUM slot (row-group-concurrent matmuls, contraction 64).
    3 slots (banks 0-5) rotate; ScalarE exp's TWO slots per ACTIVATE
    (2048 wide, via a 3D AP, negative-stride for the wrap pattern) to
    amortize the ~313-cycle ACT overhead.  PV accumulates in banks 6-7.
  - Normalization: denominators evicted to fp16 SBUF; 1/d via int16
    magic-subtract seed + one fp16 Newton step (beats the DVE's 8
    cycle/element iterative reciprocal ~3x); O * (1/d) in fp16.
  - Output projection accumulates head-pairs in banks 6/7 (after PV is
    evicted), evicts fp16, DMAs fp16 partials out (halves DMA bytes).
"""

import numpy as np

D = 1024          # d_model
T = 2048          # sequence length
G = 256           # features per head-group (4 heads * 64)
DK = 64
NKT = D // 128    # 8 contraction tiles for projections
NTT = T // 128    # 16 key tiles
NCH = T // 512    # 4 query chunks of 512
VROW = 2 * 192    # V_aug row per key tile: 2 blocks of [V_e|ones64|V_o]
MAGIC = 0x7798    # fp16 reciprocal seed: bitcast(MAGIC - bits16(d))
# fp16 Schraudolph exp for the DVE half: bitcast16(rint(s*EXP_A + EXP_B))
# ~= exp(s/8), max rel err ~3% pointwise, ~6.5e-3 end-to-end (softmax
# weights are consistent: the denominator sums the same approximated p).
EXP_A = 0.125 * 1.4426950408889634 * 1024.0
EXP_B = 15360.0 - 44.5

_CACHE = {}


def _split_multi_waits(nc):
    """walrus's TRN2 codegen rejects >1 sync-wait on datapath instruction
    structs.  Hoist every wait of a multi-wait datapath instruction onto
    single-wait NoOps just before it on the same engine queue."""
    import concourse.mybir as mybir

    keep = ("InstEventSemaphore", "InstUnconditionalBranch",
            "InstCall", "InstBranchHint", "InstHalt", "InstNoOp",
            "InstAllEngineBarrier", "InstCompareAndBranch")
    nid = [0]
    for f in nc.m.functions:
        for bb in f.blocks:
            new = []
            for ins in bb.instructions:
                si = ins.sync_info
                waits = list(si.on_wait) if si and si.on_wait else []
                if len(waits) >= 2 and type(ins).__name__ not in keep:
                    for w in waits:
                        nid[0] += 1
                        nop = mybir.InstNoOp(name=f"{ins.name}-wsplit{nid[0]}",
                                             ins=[], outs=[])
                        nop.engine = ins.engine
                        nop.sync_info = mybir.SyncInfo(on_wait=[w], on_update=[])
                        new.append(nop)
                    ins.sync_info = mybir.SyncInfo(
                        on_wait=[], on_update=list(si.on_update or []))
                new.append(ins)
            bb.instructions = new
    return nc


def _build(split_waits=True):
    import concourse.bass as bass
    import concourse.mybir as mybir
    import concourse.tile as tile
    import bass_rust

    f32 = mybir.dt.float32
    f16 = mybir.dt.float16
    i16 = mybir.dt.int16
    ALU = mybir.AluOpType
    EXP = mybir.ActivationFunctionType.Exp
    CPY = mybir.ActivationFunctionType.Copy
    nc = bass.Bass()

    xT = nc.dram_tensor("xT", [D, T], f16, kind="ExternalInput")
    wqT = nc.dram_tensor("wqT", [D, G], f16, kind="ExternalInput")
    wkT = nc.dram_tensor("wkT", [D, G], f16, kind="ExternalInput")
    wvT = nc.dram_tensor("wvT", [D, G], f16, kind="ExternalInput")
    woT = nc.dram_tensor("woT", [G, D], f16, kind="ExternalInput")
    bqc = nc.dram_tensor("bqc", [128, 2], f32, kind="ExternalInput")
    out = nc.dram_tensor("out", [T, D], f16, kind="ExternalOutput")

    with tile.TileContext(nc) as tc:
        with tc.tile_pool(name="sb", bufs=1) as sb, \
             tc.tile_pool(name="dyn", bufs=2) as dyn, \
             tc.tile_pool(name="ps", bufs=1, space="PSUM") as ps:

            # ---- the one PSUM tensor: 8 banks, manual ranges ----
            sf = ps.tile([128, 4096], f32, tag="sf", name="sf")

            # ---- PE prewarm: junk matmuls during the DMA wait keep the
            # HAM activity window busy so the real projections start at
            # 2.4 GHz instead of paying ~3.4us of 1.2 GHz cold ramp.
            # They write the pv area (banks 6/7), which the first real PV
            # overwrites with start=True.
            junk = sb.tile([128, 256], f16, tag="junk", name="junk")
            nc.gpsimd.memset(junk, 0.5)
            for _ in range(16):
                nc.tensor.matmul(out=sf[:, 3072:3328], lhsT=junk[:, 0:128],
                                 rhs=junk, start=True, stop=True)

            # ---- DMAs in consumption order ----
            wk_sb, xt = [], []
            for k in range(NKT):
                t = sb.tile([128, G], f16, tag=f"wk{k}", name=f"wk{k}")
                nc.sync.dma_start(out=t, in_=wkT[k * 128:(k + 1) * 128, :])
                wk_sb.append(t)
                t = sb.tile([128, T], f16, tag=f"xt{k}", name=f"xt{k}")
                nc.sync.dma_start(out=t, in_=xT[k * 128:(k + 1) * 128, :])
                xt.append(t)
            bq_sb = sb.tile([128, 2], f32, tag="bq", name="bq_sb")
            nc.sync.dma_start(out=bq_sb, in_=bqc[:, :])
            # warm the ScalarE exp table-set (~2.7us) during the DMA wait
            scr = sb.tile([128, 2], f16, tag="scr", name="scr")
            nc.scalar.activation(out=scr, in_=bq_sb, func=EXP, scale=0.0)
            # wq before wv: the chunk-0 Q chains run right after K so the
            # attention (and its exp streams) can start ~14us earlier
            wv_sb, wq_sb = [], []
            for nm, dram, lst in (("wq", wqT, wq_sb), ("wv", wvT, wv_sb)):
                for k in range(NKT):
                    t = sb.tile([128, G], f16, tag=f"{nm}{k}", name=f"{nm}{k}")
                    nc.sync.dma_start(out=t, in_=dram[k * 128:(k + 1) * 128, :])
                    lst.append(t)
            wo_sb = []
            for p2 in range(2):
                t = sb.tile([128, D], f16, tag=f"wo{p2}", name=f"wo{p2}")
                nc.sync.dma_start(out=t, in_=woT[p2 * 128:(p2 + 1) * 128, :])
                wo_sb.append(t)

            # ---- persistent SBUF ----
            qt = [sb.tile([128, T], f16, tag=f"qt{p}", name=f"qt{p}")
                  for p in range(2)]
            kt = [sb.tile([128, T], f16, tag=f"kt{p}", name=f"kt{p}")
                  for p in range(2)]
            va = sb.tile([128, NTT * VROW], f16, tag="va", name="va")
            va6 = va.rearrange("p (t b x) -> p t b x", t=NTT, b=6)
            nc.vector.memset(va6[:, :, 1::3, :], 1.0)   # ones64 columns

            # PSUM map: scores ring = 2 slots (banks 0-3), O-proj has its
            # OWN bank pair (4/5) so it never collides with PV (6/7).
            # The 3-slot ring gave ~1.5 units of lookahead but forced
            # O-proj to time-multiplex the PV banks, serializing
            # norm -> O-proj -> PV behind per-bank evicts at every chunk
            # boundary (~3.3us/chunk of PE bubbles).
            sf3 = sf[:, 0:2048].rearrange("p (s x) -> p s x", s=2)
            OPB = 2048          # O-proj bank pair base column
            pv_e = sf[:, 3072:3584]
            pv_o = sf[:, 3584:4096]

            def chain(i):       # 8 full-bank projection chains
                return sf[:, i * 512:(i + 1) * 512]

            # ---- K.T projection: chains (p2, c), k-outer ----
            for k in range(NKT):
                for i in range(8):
                    p2, c = divmod(i, 4)
                    nc.tensor.matmul(
                        out=chain(i),
                        lhsT=wk_sb[k][:, p2 * 128:(p2 + 1) * 128],
                        rhs=xt[k][:, c * 512:(c + 1) * 512],
                        start=(k == 0), stop=(k == NKT - 1))
            for i in range(8):
                p2, c = divmod(i, 4)
                nc.vector.tensor_copy(
                    out=kt[p2][:, c * 512:(c + 1) * 512], in_=chain(i))

            # ---- V and Q projections: only chunk-0's two Q chains and
            # the first 4 V tiles run before attention; the rest dribble
            # into chunk-0's attention units as PE filler, accumulating
            # in the O-proj banks (4/5), which chunk-0 never touches
            # (chunk-1's O-proj dribble starts at unit 17).  This starts
            # the exp streams ~14us earlier and converts chunk-0's
            # 2-slot-ring stalls into projection work.
            va5 = va.rearrange("p (t pr b x) -> p t pr b x", t=NTT, pr=2, b=3)

            def emit_qchain(c, p2, base):
                for k in range(NKT):
                    nc.tensor.matmul(
                        out=sf[:, base:base + 512],
                        lhsT=wq_sb[k][:, p2 * 128:(p2 + 1) * 128],
                        rhs=xt[k][:, c * 512:(c + 1) * 512],
                        start=(k == 0), stop=(k == NKT - 1))
                nc.vector.tensor_scalar(
                    out=qt[p2][:, c * 512:(c + 1) * 512],
                    in0=sf[:, base:base + 512],
                    scalar1=bq_sb[:, p2:p2 + 1], scalar2=None, op0=ALU.add)

            def emit_vtt(tt, base):
                for k in range(NKT):
                    nc.tensor.matmul(
                        out=sf[:, base:base + G],
                        lhsT=xt[k][:, tt * 128:(tt + 1) * 128],
                        rhs=wv_sb[k][:, :],
                        start=(k == 0), stop=(k == NKT - 1))
                nc.vector.tensor_copy(
                    out=va5[:, tt, :, 0::2, :],
                    in_=sf[:, base:base + 256].rearrange(
                        "p (pr h x) -> p pr h x", pr=2, h=2))

            emit_qchain(0, 0, 0)
            emit_qchain(0, 1, 512)
            for tt in range(4):
                emit_vtt(tt, OPB + (tt % 4) * 256)

            def emit_filler(u):
                # V tts 4-15 at units 0-5 (2-unit lead over their PV
                # consumers), Q chains c1-3 at units 6-11 (5+ units
                # before chunk 1 needs qt)
                if u < 6:
                    for tt in (4 + 2 * u, 5 + 2 * u):
                        emit_vtt(tt, OPB + (tt % 4) * 256)
                elif u < 12:
                    i = u - 6
                    emit_qchain(1 + i // 2, i % 2, OPB + (i % 2) * 512)

            # ---- attention + output projection ----
            # Flat software pipeline over 64 "units" (one unit = 2 key
            # tiles of one (chunk, head-pair)).  Per-engine queues are
            # strict FIFO, so emission order IS the schedule skeleton:
            # scores(u+1) must be emitted BEFORE pv(u) (which waits on
            # exp(u)) or the PE queue head blocks and the whole loop
            # serializes; O-proj is emitted 2 units after its chunk ends
            # so the DVE normalization latency is hidden.
            units = [(c, pair, g) for c in range(NCH) for pair in range(2)
                     for g in range(8)]
            onorm = {}          # (c, pair) -> normalized O tile

            def emit_scores(u):
                c, pair, g = units[u]
                cs = slice(c * 512, (c + 1) * 512)
                sA, sB = 0, 1
                for tk, s in ((2 * g, sA), (2 * g + 1, sB)):
                    for h in range(2):   # packed row-group pair
                        nc.tensor.matmul(
                            out=sf[:, s * 1024 + h * 512:
                                   s * 1024 + (h + 1) * 512],
                            lhsT=kt[pair][h * 64:(h + 1) * 64,
                                          tk * 128:(tk + 1) * 128],
                            rhs=qt[pair][h * 64:(h + 1) * 64, cs],
                            start=True, stop=True)

            def emit_exp(u):
                # tkA: exact exp on ScalarE; tkB: Schraudolph fast-exp on
                # the DVE (one fused mul-add into int16, bitcast to fp16).
                # The two slots live in different PSUM banks, so the two
                # engines stream concurrently - the slot-pair latency is
                # max(S, V), which is what keeps the 3-slot ring moving.
                c, pair, g = units[u]
                sA, sB = 0, 1
                pa = dyn.tile([128, 1024], f16, tag="pa", bufs=12,
                              name=f"pa_{c}_{pair}_{g}")
                nc.scalar.activation(out=pa, in_=sf3[:, sA, :], func=EXP,
                                     scale=0.125)
                if u % 16 in (4, 10):
                    # rebalance: the DVE also carries the Newton chains
                    # and half the evicts, so 2 units per chunk put BOTH
                    # halves on ScalarE (exact exp - also trims the
                    # Schraudolph error a little).  The {4,10} placement
                    # is load-bearing: moving these to the drain region
                    # {8,12} measured +48us - the held-PV bursts at
                    # in_chunk 7-9 are timed against single-exp pa
                    # completion there.
                    pb2 = dyn.tile([128, 1024], f16, tag="pbS", bufs=3,
                                   name=f"pbS_{c}_{pair}_{g}")
                    nc.scalar.activation(out=pb2, in_=sf3[:, sB, :],
                                         func=EXP, scale=0.125)
                    return pa, pb2
                pb = dyn.tile([128, 1024], i16, tag="pb", bufs=12,
                              name=f"pb_{c}_{pair}_{g}")
                nc.vector.tensor_scalar(
                    out=pb, in0=sf3[:, sB, :], scalar1=EXP_A, scalar2=EXP_B,
                    op0=ALU.mult, op1=ALU.add)
                return pa, pb.bitcast(f16)

            def emit_pv(u, pab):
                c, pair, g = units[u]
                for j in range(2):
                    tk = 2 * g + j
                    off = tk * VROW + pair * 192
                    nc.tensor.matmul(
                        out=pv_e, lhsT=va[:, off:off + 128],
                        rhs=pab[j][:, 0:512],
                        start=(tk == 0), stop=(tk == NTT - 1))
                    nc.tensor.matmul(
                        out=pv_o, lhsT=va[:, off + 64:off + 192],
                        rhs=pab[j][:, 512:1024],
                        start=(tk == 0), stop=(tk == NTT - 1))

            def emit_norm(c, pair):
                # pv_e = [O_e; d_e], pv_o = [d_o; O_o]; 1/d via int16
                # magic seed + one fp16 Newton step.  Base-aligned O
                # evicts on ScalarE, cross-base denominator evicts on the
                # DVE (both need the PSUM port).  The magic subtract is
                # SBUF-only, so it goes to the otherwise-idle GpSimd for
                # chunks 0-2; the last chunk keeps it on ScalarE so the
                # tail latency stays short.
                oo = dyn.tile([128, 512], f16, tag="oo", name=f"oo{c}{pair}")
                dd = dyn.tile([128, 512], f16, tag="dd", name=f"dd{c}{pair}")
                nc.scalar.activation(out=oo[0:64, :], in_=pv_e[0:64, :],
                                     func=CPY)
                nc.scalar.activation(out=oo[64:128, :], in_=pv_o[64:128, :],
                                     func=CPY)
                nc.vector.tensor_copy(out=dd[0:64, :], in_=pv_e[64:128, :])
                nc.vector.tensor_copy(out=dd[64:128, :], in_=pv_o[0:64, :])
                r0 = dyn.tile([128, 512], i16, tag="r0", name=f"r0{c}{pair}")
                nc.scalar.activation(out=r0, in_=dd.bitcast(i16),
                                     func=CPY, scale=-1.0,
                                     bias=float(MAGIC))
                norm_st[(c, pair)] = (oo, dd, r0)

            def emit_norm2(c, pair):
                # Newton step + final multiply on the DVE.  (Both the
                # gpsimd offload and a scalar_tensor_tensor fusion were
                # tried and lost: gpsimd's 1.15us/op serial chain and
                # STT's 1x-only mode (691ns vs 293/327 for ts/tt) are
                # slower than this 4-op 2x-mode chain.)
                oo, dd, r0 = norm_st.pop((c, pair))
                eng = nc.vector
                r = r0.bitcast(f16)
                tn = dyn.tile([128, 512], f16, tag="tn", name=f"tn{c}{pair}")
                eng.tensor_tensor(out=tn, in0=dd, in1=r, op=ALU.mult)
                un = dyn.tile([128, 512], f16, tag="un", name=f"un{c}{pair}")
                eng.tensor_scalar(
                    out=un, in0=tn, scalar1=-1.0, scalar2=2.0,
                    op0=ALU.mult, op1=ALU.add)
                r1 = dyn.tile([128, 512], f16, tag="r1", name=f"r1{c}{pair}")
                eng.tensor_tensor(out=r1, in0=r, in1=un, op=ALU.mult)
                on = dyn.tile([128, 512], f16, tag=f"on{pair}",
                              name=f"on{c}{pair}")
                eng.tensor_tensor(out=on, in0=oo, in1=r1, op=ALU.mult)
                onorm[(c, pair)] = on

            def emit_oproj(c, mts=range(4), final=False, phase=None):
                # Mid-kernel: banks 6/7 only (scores own 0-5); per-512
                # evicts alternate engines so the next 512-slot's matmuls
                # never wait a both-bank evict.  Final chunk: scores are
                # done, so each mt gets its own bank PAIR - all 16 matmuls
                # run back-to-back and the [128,1024] evicts + full-row
                # DMAs pipeline behind them.  phase=0 emits only the
                # pair-0 accumulation matmuls (they ride the pair-1 norm
                # latency); phase=1 closes with pair-1 + evict + DMA.
                for mt in mts:
                    base = mt * 1024 if final else OPB
                    for n2 in range(2):
                        ops = sf[:, base + n2 * 512:base + (n2 + 1) * 512]
                        prs = (0, 1) if phase is None else (phase,)
                        for pair in prs:
                            nc.tensor.matmul(
                                out=ops,
                                lhsT=onorm[(c, pair)][:, mt * 128:(mt + 1) * 128],
                                rhs=wo_sb[pair][:, n2 * 512:(n2 + 1) * 512],
                                start=(pair == 0), stop=(pair == 1))
                        if phase == 0:
                            continue
                        if not final:
                            osb = dyn.tile([128, 512], f16, tag="osb",
                                           bufs=4, name=f"osb_{c}_{mt}_{n2}")
                            if n2 == 0:
                                nc.scalar.activation(out=osb, in_=ops,
                                                     func=CPY)
                            else:
                                nc.vector.tensor_copy(out=osb, in_=ops)
                            nc.sync.dma_start(
                                out=out[c * 512 + mt * 128:
                                        c * 512 + (mt + 1) * 128,
                                        n2 * 512:(n2 + 1) * 512],
                                in_=osb)
                    if final and phase != 0:
                        osb = dyn.tile([128, 1024], f16, tag="osbf",
                                       bufs=4, name=f"osbf_{c}_{mt}")
                        if mt % 2 == 0:
                            nc.scalar.activation(
                                out=osb, in_=sf[:, base:base + 1024],
                                func=CPY)
                        else:
                            nc.vector.tensor_copy(
                                out=osb, in_=sf[:, base:base + 1024])
                        # one dma_start per mt (each is a ~600ns serial
                        # DIRECT2D on the Sync queue) - except the LAST-
                        # DISPATCHED tile (mt2: with mt3 hoisted first,
                        # mt2's evict trails the S queue), whose drain IS
                        # the kernel end: 2-way split halves its
                        # single-queue descriptor drain
                        if mt == 2:
                            nc.sync.dma_start(
                                out=out[c * 512 + mt * 128:
                                        c * 512 + mt * 128 + 64, :],
                                in_=osb[0:64, :])
                            nc.sync.dma_start(
                                out=out[c * 512 + mt * 128 + 64:
                                        c * 512 + (mt + 1) * 128, :],
                                in_=osb[64:128, :])
                        else:
                            nc.sync.dma_start(
                                out=out[c * 512 + mt * 128:
                                        c * 512 + (mt + 1) * 128, :],
                                in_=osb)

            # Emission = per-engine FIFO order.  Skews:
            #  - scores/exp of u+1 before pv(u), so the PE computes the
            #    next unit's scores while ScalarE exp's unit u.
            #  - With O-proj on its own bank pair, only the PAIR handoff
            #    needs a hold: each pair's first PV (start=True on banks
            #    6/7) is delayed one unit so the previous pair's oo/dd
            #    evicts clear in shadow.
            #  - O-proj(c-1) mts dribble into the two transition bubbles
            #    (chunk entry in_chunk 1-2, pair handoff 8-9) as PE
            #    filler.
            pas = {}
            pv_hold = []
            norm_st = {}
            norm_due = None
            norm2_due = None
            emit_scores(0)
            pas[0] = emit_exp(0)
            for u in range(len(units)):
                if u + 1 < len(units):
                    emit_scores(u + 1)
                    pas[u + 1] = emit_exp(u + 1)
                if norm2_due is not None:
                    # Newton chain one unit after the evicts: each engine's
                    # FIFO carries only a slice of the norm between exps
                    emit_norm2(*norm2_due)
                    norm2_due = None
                if norm_due is not None and norm_due[2] not in pas:
                    # skew the normalization late (and always after its
                    # pair's closing pv) so its ScalarE/DVE ops never block
                    # the exp stream at the queue head
                    emit_norm(norm_due[0], norm_due[1])
                    norm2_due = (norm_due[0], norm_due[1])
                    norm_due = None
                c, pair, g = units[u]
                in_chunk = u % 16
                if c > 0 and in_chunk in (1, 2, 8, 9):
                    emit_oproj(c - 1, mts=[{1: 0, 2: 1, 8: 2, 9: 3}[in_chunk]])
                if g == 0 and u >= 2:
                    pv_hold.append(u)
                else:
                    for uh in pv_hold:
                        emit_pv(uh, pas.pop(uh))
                    pv_hold = []
                    emit_pv(u, pas.pop(u))
                emit_filler(u)
                if g == 7:
                    norm_due = (c, pair, u)
            if norm2_due is not None:
                emit_norm2(*norm2_due)
            emit_norm(norm_due[0], norm_due[1])
            # keep the PE's HAM activity window busy through the ~2us
            # final-norm latency so the closing O-proj burst runs at
            # 2.4 GHz (it measured 427ns/MM = 1.2 GHz when the idle gap
            # crossed the re-throttle window)...
            for _ in range(4):
                nc.tensor.matmul(out=sf[:, 1024:1280], lhsT=junk[:, 0:128],
                                 rhs=junk, start=True, stop=True)
            # ...and fill the rest of that window with REAL work: the
            # pair-0 O-proj accumulation for mts 0-2 (banks 0-5; mt3 is
            # on the PV banks, blocked until pair-1's oo/dd)
            emit_oproj(NCH - 1, mts=range(3), final=True, phase=0)
            # bridge the remaining norm-wait idle so HAM stays at 8/8
            # (the closing burst measured 427ns/MM = 1.2 GHz without
            # this).  Target the PV banks: ordered AFTER the oo/dd
            # evicts (WAR) and BEFORE mt3's start=True overwrite - NOT
            # banks 0-5, which hold the phase-0 partial sums.
            for _ in range(10):
                nc.tensor.matmul(out=sf[:, 3072:3328], lhsT=junk[:, 0:128],
                                 rhs=junk, start=True, stop=True)
            emit_norm2(norm_due[0], norm_due[1])
            # mt3 first: it only needs banks 6/7 (clear since oo/dd) and
            # its evict engine (DVE) is idle right after the norm chain,
            # so its 256KB starts draining ~1.5us earlier
            emit_oproj(NCH - 1, mts=[3], final=True)
            emit_oproj(NCH - 1, mts=range(3), final=True, phase=1)
    if split_waits:
        _split_multi_waits(nc)
    return nc


def _get_nc(split_waits=True):
    key = ("nc", split_waits)
    if key not in _CACHE:
        _CACHE[key] = _build(split_waits)
    return _CACHE[key]


def make_in_maps(x, Wq, bq, Wk, bk, Wv, bv, Wo):
    dt = np.float16
    in_maps = []
    for core in range(8):
        b, g = divmod(core, 4)
        gs = slice(g * G, (g + 1) * G)
        in_maps.append({
            "xT": np.ascontiguousarray(x[b].T).astype(dt),
            "wqT": np.ascontiguousarray(Wq[gs, :].T).astype(dt),
            "wkT": np.ascontiguousarray(Wk[gs, :].T).astype(dt),
            "wvT": np.ascontiguousarray(Wv[gs, :].T).astype(dt),
            "woT": np.ascontiguousarray(Wo[:, gs].T).astype(dt),
            "bqc": np.ascontiguousarray(bq[gs].reshape(2, 128).T).astype(np.float32),
        })
    return in_maps


def host_out_init(bo, bv, Wo):
    """bo + bv @ Wo.T (the bv contribution is exact: softmax rows sum to 1)."""
    return (bo.astype(np.float64)
            + bv.astype(np.float64) @ Wo.T.astype(np.float64)).astype(np.float32)


def kernel(x, Wq, bq, Wk, bk, Wv, bv, Wo, bo):
    from concourse.bass_utils import run_bass_kernel_spmd

    x = np.asarray(x, dtype=np.float32)
    Wq = np.asarray(Wq, dtype=np.float32)
    Wk = np.asarray(Wk, dtype=np.float32)
    Wv = np.asarray(Wv, dtype=np.float32)
    Wo = np.asarray(Wo, dtype=np.float32)
    bq = np.asarray(bq, dtype=np.float32)
    bv = np.asarray(bv, dtype=np.float32)
    bo = np.asarray(bo, dtype=np.float32)

    nc = _get_nc()
    in_maps = make_in_maps(x, Wq, bq, Wk, bk, Wv, bv, Wo)

    res = run_bass_kernel_spmd(nc, in_maps, core_ids=list(range(8)))
    outp = np.tile(host_out_init(bo, bv, Wo)[None, None, :], (2, T, 1))
    for core in range(8):
        outp[core // 4] += res.results[core]["out"].astype(np.float32)
    return outp



# revision 62
# speedup vs baseline: 1.0200x; 1.0046x over previous
"""Multi-head attention Trainium2 Bass kernel (v4.4). 197.3us
(v2 baseline: 208.9us; all numbers at full clock, min-MM 215ns).

v4.x over v3:
  - PSUM remap: scores ring = 2 slots (banks 0-3), O-proj owns banks
    4/5, PV owns 6/7.  O-proj never collides with PV, so the old
    hold/drain/dribble machinery shrank to a 1-unit pair-handoff hold
    + a 4-mt dribble; chunk-boundary PE bubbles fell ~3.3->1us/chunk
    (PE 94.5% busy in attention).  The cost: scores(u+1) waits
    exp(u) directly (2-slot ring) - with 8 banks you get scores
    lookahead OR O-proj isolation, not both.
  - Projection interleave: only K, the chunk-0 Q chains, and 4 V
    tiles run before attention; V tts 4-15 and Q chains c1-3 dribble
    into chunk-0's units as PE filler, accumulating in the then-idle
    O-proj banks.  First scores at 32us instead of 56us; the
    attention-start and chunk-0/1 boundary gaps vanished.
  - PE prewarm (20 junk matmuls) + exp rebalance (2 units/chunk with
    both slots exact on ScalarE, at in_chunk {4,10} - load-bearing
    placement) + final-chunk O-proj over all 4 bank pairs with
    phase-0 pair-0 matmuls and junk bridges riding the closing norm
    latency (burst measured warm at 216ns/MM) + 2-way split of only
    the last out-DMA.
Measured dead ends (reverted): fp8/DoubleRow anywhere (e4m3 noise
blows the 2e-2 gate: PV 1.9e-2, proj 3.0e-2, O-proj 2.8e-2 emulated);
2048-wide merged exp ops (couple the two PSUM slots -> ring stalls,
+28us); per-unit engine alternation for exp (serializes slot frees,
+22us); GpSimd Newton chain (1.15us/op serial chain parks an O-proj
matmul at the PE queue head ~2.3us/chunk); splitting DMAs across
queues at the tail (each dma_start is a ~600ns serial Sync-queue
DIRECT2D); splitting the first x-tile DMAs (descriptor inflation
starves the K-projection); scalar_tensor_tensor Newton fusion (STT
is 1x-mode-only: 691ns vs the 293/327ns 2x ts/tt it replaces);
double-S exp units in the drain region (+48us incl thermal).

Bench note: back-to-back runs heat the chip into P0 (PE 2.4->2.0
GHz, warm N=512 matmul 215->258ns, ~+17% exec).  Compare only runs
with matching min-matmul durations.

Remaining structural gap (~190us floor): the chunk-boundary bubbles
(~3.3us/chunk) come from 8-bank PSUM scarcity - scores need 6, PV 2,
and O-proj reuses the PV pair, serializing norm -> O-proj -> PV
behind per-bank evicts.  A fix needs 10 banks or 512-query chunks.

Problem: B=2, T=2048, D=1024, H=16 heads, dk=64 (fp32).
  out = softmax((x@Wq.T+bq)(x@Wk.T+bk).T / 8) (x@Wv.T+bv) @ Wo.T + bo

Sharding (8 cores): data-parallel over B (2) x tensor-parallel over 4
head-groups of 4 heads.  Core (b, g) computes, for batch b and heads
[4g, 4g+4): Q/K/V projections (column-sliced Wq/Wk/Wv), attention, and
the row-sliced Wo projection, producing a partial (2048, 1024) fp16
output.  Host sums the partials per batch in fp32 and adds the bias
terms.

Bias algebra (removes all device-side bias work except bq):
  - bk shifts every score of a query by a constant -> softmax-invariant
    -> dropped entirely.
  - bv: softmax rows sum to 1, so the bv contribution to the output is
    the constant row bv @ Wo.T -> folded into bo on the host.
  - bq: added on the Q-projection eviction via a per-partition
    tensor_scalar add (Q.T layout has features on partitions).

Per-core device schedule (everything fp16 operands, fp32 PSUM):
  - One persistent PSUM tensor sf [128, 4096] (all 8 banks) managed
    manually with subtile dependency tracking - no pool barriers, so
    the scheduler freely overlaps phases.
  - Projections (k-outer, 8 full-bank chains): K.T -> V -> Q.T, each
    chain accumulates 8 k-tiles; DMAs are issued in consumption order
    so the PE starts as soon as wk0+xt0 land.
  - V stored as V_aug [128, 16*384]: per key-tile, per head-pair block
    [V_even|ones64|V_odd] so the PV matmul also produces the softmax
    denominator (replicated across 64 partitions) for free.
  - Attention per (chunk c of 512 queries, head-pair):  scores.T tiles
    [128 keys, 512q] per head, both heads of the pair packed into one
    1024-wide PSUM slot (row-group-concurrent matmuls, contraction 64).
    3 slots (banks 0-5) rotate; ScalarE exp's TWO slots per ACTIVATE
    (2048 wide, via a 3D AP, negative-stride for the wrap pattern) to
    amortize the ~313-cycle ACT overhead.  PV accumulates in banks 6-7.
  - Normalization: denominators evicted to fp16 SBUF; 1/d via int16
    magic-subtract seed + one fp16 Newton step (beats the DVE's 8
    cycle/element iterative reciprocal ~3x); O * (1/d) in fp16.
  - Output projection accumulates head-pairs in banks 6/7 (after PV is
    evicted), evicts fp16, DMAs fp16 partials out (halves DMA bytes).
"""

import numpy as np

D = 1024          # d_model
T = 2048          # sequence length
G = 256           # features per head-group (4 heads * 64)
DK = 64
NKT = D // 128    # 8 contraction tiles for projections
NTT = T // 128    # 16 key tiles
NCH = T // 512    # 4 query chunks of 512
VROW = 2 * 192    # V_aug row per key tile: 2 blocks of [V_e|ones64|V_o]
MAGIC = 0x7798    # fp16 reciprocal seed: bitcast(MAGIC - bits16(d))
# fp16 Schraudolph exp for the DVE half: bitcast16(rint(s*EXP_A + EXP_B))
# ~= exp(s/8), max rel err ~3% pointwise, ~6.5e-3 end-to-end (softmax
# weights are consistent: the denominator sums the same approximated p).
EXP_A = 0.125 * 1.4426950408889634 * 1024.0
EXP_B = 15360.0 - 44.5

_CACHE = {}


def _split_multi_waits(nc):
    """walrus's TRN2 codegen rejects >1 sync-wait on datapath instruction
    structs.  Hoist every wait of a multi-wait datapath instruction onto
    single-wait NoOps just before it on the same engine queue."""
    import concourse.mybir as mybir

    keep = ("InstEventSemaphore", "InstUnconditionalBranch",
            "InstCall", "InstBranchHint", "InstHalt", "InstNoOp",
            "InstAllEngineBarrier", "InstCompareAndBranch")
    nid = [0]
    for f in nc.m.functions:
        for bb in f.blocks:
            new = []
            for ins in bb.instructions:
                si = ins.sync_info
                waits = list(si.on_wait) if si and si.on_wait else []
                if len(waits) >= 2 and type(ins).__name__ not in keep:
                    for w in waits:
                        nid[0] += 1
                        nop = mybir.InstNoOp(name=f"{ins.name}-wsplit{nid[0]}",
                                             ins=[], outs=[])
                        nop.engine = ins.engine
                        nop.sync_info = mybir.SyncInfo(on_wait=[w], on_update=[])
                        new.append(nop)
                    ins.sync_info = mybir.SyncInfo(
                        on_wait=[], on_update=list(si.on_update or []))
                new.append(ins)
            bb.instructions = new
    return nc


def _build(split_waits=True):
    import concourse.bass as bass
    import concourse.mybir as mybir
    import concourse.tile as tile
    import bass_rust

    f32 = mybir.dt.float32
    f16 = mybir.dt.float16
    i16 = mybir.dt.int16
    ALU = mybir.AluOpType
    EXP = mybir.ActivationFunctionType.Exp
    CPY = mybir.ActivationFunctionType.Copy
    nc = bass.Bass()

    xT = nc.dram_tensor("xT", [D, T], f16, kind="ExternalInput")
    wqT = nc.dram_tensor("wqT", [D, G], f16, kind="ExternalInput")
    wkT = nc.dram_tensor("wkT", [D, G], f16, kind="ExternalInput")
    wvT = nc.dram_tensor("wvT", [D, G], f16, kind="ExternalInput")
    woT = nc.dram_tensor("woT", [G, D], f16, kind="ExternalInput")
    bqc = nc.dram_tensor("bqc", [128, 2], f32, kind="ExternalInput")
    out = nc.dram_tensor("out", [T, D], f16, kind="ExternalOutput")

    with tile.TileContext(nc) as tc:
        with tc.tile_pool(name="sb", bufs=1) as sb, \
             tc.tile_pool(name="dyn", bufs=2) as dyn, \
             tc.tile_pool(name="ps", bufs=1, space="PSUM") as ps:

            # ---- the one PSUM tensor: 8 banks, manual ranges ----
            sf = ps.tile([128, 4096], f32, tag="sf", name="sf")

            # ---- PE prewarm: junk matmuls during the DMA wait keep the
            # HAM activity window busy so the real projections start at
            # 2.4 GHz instead of paying ~3.4us of 1.2 GHz cold ramp.
            # They write the pv area (banks 6/7), which the first real PV
            # overwrites with start=True.
            junk = sb.tile([128, 256], f16, tag="junk", name="junk")
            nc.gpsimd.memset(junk, 0.5)
            for _ in range(16):
                nc.tensor.matmul(out=sf[:, 3072:3328], lhsT=junk[:, 0:128],
                                 rhs=junk, start=True, stop=True)

            # ---- DMAs in consumption order ----
            wk_sb, xt = [], []
            for k in range(NKT):
                t = sb.tile([128, G], f16, tag=f"wk{k}", name=f"wk{k}")
                nc.sync.dma_start(out=t, in_=wkT[k * 128:(k + 1) * 128, :])
                wk_sb.append(t)
                t = sb.tile([128, T], f16, tag=f"xt{k}", name=f"xt{k}")
                nc.sync.dma_start(out=t, in_=xT[k * 128:(k + 1) * 128, :])
                xt.append(t)
            bq_sb = sb.tile([128, 2], f32, tag="bq", name="bq_sb")
            nc.sync.dma_start(out=bq_sb, in_=bqc[:, :])
            # warm the ScalarE exp table-set (~2.7us) during the DMA wait
            scr = sb.tile([128, 2], f16, tag="scr", name="scr")
            nc.scalar.activation(out=scr, in_=bq_sb, func=EXP, scale=0.0)
            # wq before wv: the chunk-0 Q chains run right after K so the
            # attention (and its exp streams) can start ~14us earlier
            wv_sb, wq_sb = [], []
            for nm, dram, lst in (("wq", wqT, wq_sb), ("wv", wvT, wv_sb)):
                for k in range(NKT):
                    t = sb.tile([128, G], f16, tag=f"{nm}{k}", name=f"{nm}{k}")
                    nc.sync.dma_start(out=t, in_=dram[k * 128:(k + 1) * 128, :])
                    lst.append(t)
            wo_sb = []
            for p2 in range(2):
                t = sb.tile([128, D], f16, tag=f"wo{p2}", name=f"wo{p2}")
                nc.sync.dma_start(out=t, in_=woT[p2 * 128:(p2 + 1) * 128, :])
                wo_sb.append(t)

            # ---- persistent SBUF ----
            qt = [sb.tile([128, T], f16, tag=f"qt{p}", name=f"qt{p}")
                  for p in range(2)]
            kt = [sb.tile([128, T], f16, tag=f"kt{p}", name=f"kt{p}")
                  for p in range(2)]
            va = sb.tile([128, NTT * VROW], f16, tag="va", name="va")
            va6 = va.rearrange("p (t b x) -> p t b x", t=NTT, b=6)
            nc.vector.memset(va6[:, :, 1::3, :], 1.0)   # ones64 columns

            # PSUM map: scores ring = 2 slots (banks 0-3), O-proj has its
            # OWN bank pair (4/5) so it never collides with PV (6/7).
            # The 3-slot ring gave ~1.5 units of lookahead but forced
            # O-proj to time-multiplex the PV banks, serializing
            # norm -> O-proj -> PV behind per-bank evicts at every chunk
            # boundary (~3.3us/chunk of PE bubbles).
            sf3 = sf[:, 0:2048].rearrange("p (s x) -> p s x", s=2)
            OPB = 2048          # O-proj bank pair base column
            pv_e = sf[:, 3072:3584]
            pv_o = sf[:, 3584:4096]

            def chain(i):       # 8 full-bank projection chains
                return sf[:, i * 512:(i + 1) * 512]

            # ---- K.T projection: chains (p2, c), k-outer ----
            for k in range(NKT):
                for i in range(8):
                    p2, c = divmod(i, 4)
                    nc.tensor.matmul(
                        out=chain(i),
                        lhsT=wk_sb[k][:, p2 * 128:(p2 + 1) * 128],
                        rhs=xt[k][:, c * 512:(c + 1) * 512],
                        start=(k == 0), stop=(k == NKT - 1))
            for i in range(8):
                p2, c = divmod(i, 4)
                nc.vector.tensor_copy(
                    out=kt[p2][:, c * 512:(c + 1) * 512], in_=chain(i))

            # ---- V and Q projections: only chunk-0's two Q chains and
            # the first 4 V tiles run before attention; the rest dribble
            # into chunk-0's attention units as PE filler, accumulating
            # in the O-proj banks (4/5), which chunk-0 never touches
            # (chunk-1's O-proj dribble starts at unit 17).  This starts
            # the exp streams ~14us earlier and converts chunk-0's
            # 2-slot-ring stalls into projection work.
            va5 = va.rearrange("p (t pr b x) -> p t pr b x", t=NTT, pr=2, b=3)

            def emit_qchain(c, p2, base):
                for k in range(NKT):
                    nc.tensor.matmul(
                        out=sf[:, base:base + 512],
                        lhsT=wq_sb[k][:, p2 * 128:(p2 + 1) * 128],
                        rhs=xt[k][:, c * 512:(c + 1) * 512],
                        start=(k == 0), stop=(k == NKT - 1))
                nc.vector.tensor_scalar(
                    out=qt[p2][:, c * 512:(c + 1) * 512],
                    in0=sf[:, base:base + 512],
                    scalar1=bq_sb[:, p2:p2 + 1], scalar2=None, op0=ALU.add)

            def emit_vtt(tt, base):
                for k in range(NKT):
                    nc.tensor.matmul(
                        out=sf[:, base:base + G],
                        lhsT=xt[k][:, tt * 128:(tt + 1) * 128],
                        rhs=wv_sb[k][:, :],
                        start=(k == 0), stop=(k == NKT - 1))
                nc.vector.tensor_copy(
                    out=va5[:, tt, :, 0::2, :],
                    in_=sf[:, base:base + 256].rearrange(
                        "p (pr h x) -> p pr h x", pr=2, h=2))

            emit_qchain(0, 0, 0)
            emit_qchain(0, 1, 512)
            for tt in range(4):
                emit_vtt(tt, OPB + (tt % 4) * 256)

            def emit_filler(u):
                # V tts 4-15 at units 0-5 (2-unit lead over their PV
                # consumers), Q chains c1-3 at units 6-11 (5+ units
                # before chunk 1 needs qt)
                if u < 6:
                    for tt in (4 + 2 * u, 5 + 2 * u):
                        emit_vtt(tt, OPB + (tt % 4) * 256)
                elif u < 12:
                    i = u - 6
                    emit_qchain(1 + i // 2, i % 2, OPB + (i % 2) * 512)

            # ---- attention + output projection ----
            # Flat software pipeline over 64 "units" (one unit = 2 key
            # tiles of one (chunk, head-pair)).  Per-engine queues are
            # strict FIFO, so emission order IS the schedule skeleton:
            # scores(u+1) must be emitted BEFORE pv(u) (which waits on
            # exp(u)) or the PE queue head blocks and the whole loop
            # serializes; O-proj is emitted 2 units after its chunk ends
            # so the DVE normalization latency is hidden.
            units = [(c, pair, g) for c in range(NCH) for pair in range(2)
                     for g in range(8)]
            onorm = {}          # (c, pair) -> normalized O tile

            def emit_scores(u):
                c, pair, g = units[u]
                cs = slice(c * 512, (c + 1) * 512)
                sA, sB = 0, 1
                for tk, s in ((2 * g, sA), (2 * g + 1, sB)):
                    for h in range(2):   # packed row-group pair
                        nc.tensor.matmul(
                            out=sf[:, s * 1024 + h * 512:
                                   s * 1024 + (h + 1) * 512],
                            lhsT=kt[pair][h * 64:(h + 1) * 64,
                                          tk * 128:(tk + 1) * 128],
                            rhs=qt[pair][h * 64:(h + 1) * 64, cs],
                            start=True, stop=True)

            def emit_exp(u):
                # tkA: exact exp on ScalarE; tkB: Schraudolph fast-exp on
                # the DVE (one fused mul-add into int16, bitcast to fp16).
                # The two slots live in different PSUM banks, so the two
                # engines stream concurrently - the slot-pair latency is
                # max(S, V), which is what keeps the 3-slot ring moving.
                c, pair, g = units[u]
                sA, sB = 0, 1
                pa = dyn.tile([128, 1024], f16, tag="pa", bufs=12,
                              name=f"pa_{c}_{pair}_{g}")
                nc.scalar.activation(out=pa, in_=sf3[:, sA, :], func=EXP,
                                     scale=0.125)
                if u % 16 in (4, 10):
                    # rebalance: the DVE also carries the Newton chains
                    # and half the evicts, so 2 units per chunk put BOTH
                    # halves on ScalarE (exact exp - also trims the
                    # Schraudolph error a little).  The {4,10} placement
                    # is load-bearing: moving these to the drain region
                    # {8,12} measured +48us - the held-PV bursts at
                    # in_chunk 7-9 are timed against single-exp pa
                    # completion there.
                    pb2 = dyn.tile([128, 1024], f16, tag="pbS", bufs=3,
                                   name=f"pbS_{c}_{pair}_{g}")
                    nc.scalar.activation(out=pb2, in_=sf3[:, sB, :],
                                         func=EXP, scale=0.125)
                    return pa, pb2
                pb = dyn.tile([128, 1024], i16, tag="pb", bufs=12,
                              name=f"pb_{c}_{pair}_{g}")
                nc.vector.tensor_scalar(
                    out=pb, in0=sf3[:, sB, :], scalar1=EXP_A, scalar2=EXP_B,
                    op0=ALU.mult, op1=ALU.add)
                return pa, pb.bitcast(f16)

            def emit_pv(u, pab):
                c, pair, g = units[u]
                for j in range(2):
                    tk = 2 * g + j
                    off = tk * VROW + pair * 192
                    nc.tensor.matmul(
                        out=pv_e, lhsT=va[:, off:off + 128],
                        rhs=pab[j][:, 0:512],
                        start=(tk == 0), stop=(tk == NTT - 1))
                    nc.tensor.matmul(
                        out=pv_o, lhsT=va[:, off + 64:off + 192],
                        rhs=pab[j][:, 512:1024],
                        start=(tk == 0), stop=(tk == NTT - 1))

            def emit_norm(c, pair):
                # pv_e = [O_e; d_e], pv_o = [d_o; O_o]; 1/d via int16
                # magic seed + one fp16 Newton step.  Base-aligned O
                # evicts on ScalarE, cross-base denominator evicts on the
                # DVE (both need the PSUM port).  The magic subtract is
                # SBUF-only, so it goes to the otherwise-idle GpSimd for
                # chunks 0-2; the last chunk keeps it on ScalarE so the
                # tail latency stays short.
                oo = dyn.tile([128, 512], f16, tag="oo", name=f"oo{c}{pair}")
                dd = dyn.tile([128, 512], f16, tag="dd", name=f"dd{c}{pair}")
                nc.scalar.activation(out=oo[0:64, :], in_=pv_e[0:64, :],
                                     func=CPY)
                nc.scalar.activation(out=oo[64:128, :], in_=pv_o[64:128, :],
                                     func=CPY)
                nc.vector.tensor_copy(out=dd[0:64, :], in_=pv_e[64:128, :])
                nc.vector.tensor_copy(out=dd[64:128, :], in_=pv_o[0:64, :])
                r0 = dyn.tile([128, 512], i16, tag="r0", name=f"r0{c}{pair}")
                nc.scalar.activation(out=r0, in_=dd.bitcast(i16),
                                     func=CPY, scale=-1.0,
                                     bias=float(MAGIC))
                norm_st[(c, pair)] = (oo, dd, r0)

            def emit_norm2(c, pair):
                # Newton step + final multiply on the DVE.  (Both the
                # gpsimd offload and a scalar_tensor_tensor fusion were
                # tried and lost: gpsimd's 1.15us/op serial chain and
                # STT's 1x-only mode (691ns vs 293/327 for ts/tt) are
                # slower than this 4-op 2x-mode chain.)
                oo, dd, r0 = norm_st.pop((c, pair))
                eng = nc.vector
                r = r0.bitcast(f16)
                tn = dyn.tile([128, 512], f16, tag="tn", name=f"tn{c}{pair}")
                eng.tensor_tensor(out=tn, in0=dd, in1=r, op=ALU.mult)
                un = dyn.tile([128, 512], f16, tag="un", name=f"un{c}{pair}")
                eng.tensor_scalar(
                    out=un, in0=tn, scalar1=-1.0, scalar2=2.0,
                    op0=ALU.mult, op1=ALU.add)
                r1 = dyn.tile([128, 512], f16, tag="r1", name=f"r1{c}{pair}")
                eng.tensor_tensor(out=r1, in0=r, in1=un, op=ALU.mult)
                on = dyn.tile([128, 512], f16, tag=f"on{pair}",
                              name=f"on{c}{pair}")
                eng.tensor_tensor(out=on, in0=oo, in1=r1, op=ALU.mult)
                onorm[(c, pair)] = on

            def emit_oproj(c, mts=range(4), final=False, phase=None):
                # Mid-kernel: banks 6/7 only (scores own 0-5); per-512
                # evicts alternate engines so the next 512-slot's matmuls
                # never wait a both-bank evict.  Final chunk: scores are
                # done, so each mt gets its own bank PAIR - all 16 matmuls
                # run back-to-back and the [128,1024] evicts + full-row
                # DMAs pipeline behind them.  phase=0 emits only the
                # pair-0 accumulation matmuls (they ride the pair-1 norm
                # latency); phase=1 closes with pair-1 + evict + DMA.
                for mt in mts:
                    base = mt * 1024 if final else OPB
                    for n2 in range(2):
                        ops = sf[:, base + n2 * 512:base + (n2 + 1) * 512]
                        prs = (0, 1) if phase is None else (phase,)
                        for pair in prs:
                            nc.tensor.matmul(
                                out=ops,
                                lhsT=onorm[(c, pair)][:, mt * 128:(mt + 1) * 128],
                                rhs=wo_sb[pair][:, n2 * 512:(n2 + 1) * 512],
                                start=(pair == 0), stop=(pair == 1))
                        if phase == 0:
                            continue
                        if not final:
                            osb = dyn.tile([128, 512], f16, tag="osb",
                                           bufs=4, name=f"osb_{c}_{mt}_{n2}")
                            if n2 == 0:
                                nc.scalar.activation(out=osb, in_=ops,
                                                     func=CPY)
                            else:
                                nc.vector.tensor_copy(out=osb, in_=ops)
                            nc.sync.dma_start(
                                out=out[c * 512 + mt * 128:
                                        c * 512 + (mt + 1) * 128,
                                        n2 * 512:(n2 + 1) * 512],
                                in_=osb)
                    if final and phase != 0:
                        osb = dyn.tile([128, 1024], f16, tag="osbf",
                                       bufs=4, name=f"osbf_{c}_{mt}")
                        if mt % 2 == 0:
                            nc.scalar.activation(
                                out=osb, in_=sf[:, base:base + 1024],
                                func=CPY)
                        else:
                            nc.vector.tensor_copy(
                                out=osb, in_=sf[:, base:base + 1024])
                        # one dma_start per mt, no splits: the final DMA
                        # drains finish UNDER the ~2.1us NEFF postamble
                        # (build_end), which begins only after the LAST
                        # Sync-queue dispatch - an extra 600ns DIRECT2D
                        # delays the kernel end more than its halved
                        # drain saves
                        nc.sync.dma_start(
                            out=out[c * 512 + mt * 128:
                                    c * 512 + (mt + 1) * 128, :],
                            in_=osb)

            # Emission = per-engine FIFO order.  Skews:
            #  - scores/exp of u+1 before pv(u), so the PE computes the
            #    next unit's scores while ScalarE exp's unit u.
            #  - With O-proj on its own bank pair, only the PAIR handoff
            #    needs a hold: each pair's first PV (start=True on banks
            #    6/7) is delayed one unit so the previous pair's oo/dd
            #    evicts clear in shadow.
            #  - O-proj(c-1) mts dribble into the two transition bubbles
            #    (chunk entry in_chunk 1-2, pair handoff 8-9) as PE
            #    filler.
            pas = {}
            pv_hold = []
            norm_st = {}
            norm_due = None
            norm2_due = None
            emit_scores(0)
            pas[0] = emit_exp(0)
            for u in range(len(units)):
                if u + 1 < len(units):
                    emit_scores(u + 1)
                    pas[u + 1] = emit_exp(u + 1)
                if norm2_due is not None:
                    # Newton chain one unit after the evicts: each engine's
                    # FIFO carries only a slice of the norm between exps
                    emit_norm2(*norm2_due)
                    norm2_due = None
                if norm_due is not None and norm_due[2] not in pas:
                    # skew the normalization late (and always after its
                    # pair's closing pv) so its ScalarE/DVE ops never block
                    # the exp stream at the queue head
                    emit_norm(norm_due[0], norm_due[1])
                    norm2_due = (norm_due[0], norm_due[1])
                    norm_due = None
                c, pair, g = units[u]
                in_chunk = u % 16
                if c > 0 and in_chunk in (1, 2, 8, 9):
                    emit_oproj(c - 1, mts=[{1: 0, 2: 1, 8: 2, 9: 3}[in_chunk]])
                if g == 0 and u >= 2:
                    pv_hold.append(u)
                else:
                    for uh in pv_hold:
                        emit_pv(uh, pas.pop(uh))
                    pv_hold = []
                    emit_pv(u, pas.pop(u))
                emit_filler(u)
                if g == 7:
                    norm_due = (c, pair, u)
            if norm2_due is not None:
                emit_norm2(*norm2_due)
            emit_norm(norm_due[0], norm_due[1])
            # keep the PE's HAM activity window busy through the ~2us
            # final-norm latency so the closing O-proj burst runs at
            # 2.4 GHz (it measured 427ns/MM = 1.2 GHz when the idle gap
            # crossed the re-throttle window)...
            for _ in range(4):
                nc.tensor.matmul(out=sf[:, 1024:1280], lhsT=junk[:, 0:128],
                                 rhs=junk, start=True, stop=True)
            # ...and fill the rest of that window with REAL work: the
            # pair-0 O-proj accumulation for mts 0-2 (banks 0-5; mt3 is
            # on the PV banks, blocked until pair-1's oo/dd)
            emit_oproj(NCH - 1, mts=range(3), final=True, phase=0)
            # bridge the remaining norm-wait idle so HAM stays at 8/8
            # (the closing burst measured 427ns/MM = 1.2 GHz without
            # this).  Target the PV banks: ordered AFTER the oo/dd
            # evicts (WAR) and BEFORE mt3's start=True overwrite - NOT
            # banks 0-5, which hold the phase-0 partial sums.
            for _ in range(10):
                nc.tensor.matmul(out=sf[:, 3072:3328], lhsT=junk[:, 0:128],
                                 rhs=junk, start=True, stop=True)
            emit_norm2(norm_due[0], norm_due[1])
            # mt3 first: it only needs banks 6/7 (clear since oo/dd) and
            # its evict engine (DVE) is idle right after the norm chain,
            # so its 256KB starts draining ~1.5us earlier
            emit_oproj(NCH - 1, mts=[3], final=True)
            emit_oproj(NCH - 1, mts=range(3), final=True, phase=1)
    if split_waits:
        _split_multi_waits(nc)
    return nc


def _get_nc(split_waits=True):
    key = ("nc", split_waits)
    if key not in _CACHE:
        _CACHE[key] = _build(split_waits)
    return _CACHE[key]


def make_in_maps(x, Wq, bq, Wk, bk, Wv, bv, Wo):
    dt = np.float16
    in_maps = []
    for core in range(8):
        b, g = divmod(core, 4)
        gs = slice(g * G, (g + 1) * G)
        in_maps.append({
            "xT": np.ascontiguousarray(x[b].T).astype(dt),
            "wqT": np.ascontiguousarray(Wq[gs, :].T).astype(dt),
            "wkT": np.ascontiguousarray(Wk[gs, :].T).astype(dt),
            "wvT": np.ascontiguousarray(Wv[gs, :].T).astype(dt),
            "woT": np.ascontiguousarray(Wo[:, gs].T).astype(dt),
            "bqc": np.ascontiguousarray(bq[gs].reshape(2, 128).T).astype(np.float32),
        })
    return in_maps


def host_out_init(bo, bv, Wo):
    """bo + bv @ Wo.T (the bv contribution is exact: softmax rows sum to 1)."""
    return (bo.astype(np.float64)
            + bv.astype(np.float64) @ Wo.T.astype(np.float64)).astype(np.float32)


def kernel(x, Wq, bq, Wk, bk, Wv, bv, Wo, bo):
    from concourse.bass_utils import run_bass_kernel_spmd

    x = np.asarray(x, dtype=np.float32)
    Wq = np.asarray(Wq, dtype=np.float32)
    Wk = np.asarray(Wk, dtype=np.float32)
    Wv = np.asarray(Wv, dtype=np.float32)
    Wo = np.asarray(Wo, dtype=np.float32)
    bq = np.asarray(bq, dtype=np.float32)
    bv = np.asarray(bv, dtype=np.float32)
    bo = np.asarray(bo, dtype=np.float32)

    nc = _get_nc()
    in_maps = make_in_maps(x, Wq, bq, Wk, bk, Wv, bv, Wo)

    res = run_bass_kernel_spmd(nc, in_maps, core_ids=list(range(8)))
    outp = np.tile(host_out_init(bo, bv, Wo)[None, None, :], (2, T, 1))
    for core in range(8):
        outp[core // 4] += res.results[core]["out"].astype(np.float32)
    return outp

